# revision 55
# baseline (speedup 1.0000x reference)
"""Trainium2 Bass kernel for the Brill-Lindquist Christoffel-symbol grid.

Math: the reference reduces to
    psi  = 1 + sum_n m_n / (2 r_n),   m = softplus(pre)
    h    = psi^4
    G_c  = finite-difference gradient of h along grid axis c (2nd order
           central interior, 1st order one-sided edges, spacing DX)
    W_c  = 0.5 * G_c / h
    Gamma^i_{jk} = delta_ij W_k + delta_ik W_j - delta_jk W_i
so the [96,96,96,3,3,3] output carries only 3 distinct scalar fields
(W_0, W_1, W_2) per point; the 27 slots are +-W_c / 0.

Layout: partitions = a1 (96 of 128), free = (a0_ext, z) with a0 sharded
12 planes/core + 1 clamped halo plane each side (14 blocks x 96 = 1344
ext cols, 1152 interior). Pipeline per a0 chunk (4 chunks, small first
so downstream engines start early):
  - r^2/c_n by K=2 matmuls (PE), q_n = m_n/(2 r_n) via ONE ACT
    Abs_reciprocal_sqrt pass straight from PSUM (~4e-5 accurate,
    replaces the vector reciprocal + sqrt chain);
  - psi^2 = Square(q0+q1, bias=1) and h = Square(psi^2) -> bf16 (ACT);
  - H fp32 re-materialized by an identity matmul of h_bf16 on the idle
    PE (scaled by 1/C_INT), so hc = recip(PSUM) = C_INT * psi^-4 comes
    out pre-scaled on DVE with no extra squaring pass;
  - a1-FD  = ONE shared 96x96 bf16 matmul matrix (edge rows one-sided,
    all scales folded: entries +-1/+-2), W1 = PSUM * hc (DVE);
  - a0-FD  = dense free-dim shift-sub by +-96 cols (bf16, DVE 2x);
  - z-FD   = dense shift-sub by +-1 col; block-boundary cols rewritten
    by a strided one-sided fix on gpsimd (W0/W2 muls also gpsimd).
Clamped ghost planes make the global a0 one-sided edge equal to the
central formula up to a scale, supplied per core as a broadcast input
vector, so ONE program serves all 8 cores (SPMD).

Device output per core is just the 3 distinct fields, bf16, slot-packed
[a1(96), a0_loc(12) x c(3) x z(96)] = 0.66 MB (vs 95.6 MB full f32
output); the host inserts zeros/signs for the 27 slots, upcasts and
permutes, the same class of gather work as the zero-slot insertion the
problem requires anyway.
"""

import numpy as np

RES = 96
N_CORES = 8
PLANES = RES // N_CORES        # 12 a0 planes per core
EXTB = PLANES + 2              # 14 ext blocks (halo both sides)
E = EXTB * RES                 # 1344 ext free cols
I = PLANES * RES               # 1152 interior free cols
OW = PLANES * 3 * RES          # 3456 output cols per partition

_DX = float(np.float32(1.0 / (RES / 2 - 1)))
C_INT = 0.25 / _DX             # 11.75  (0.5 christoffel * central 1/(2DX))
C_EDG = 0.5 / _DX              # 23.5   (0.5 christoffel * one-sided 1/DX)

# ext chunks (blocks) for the field build and matching interior chunks;
# first chunk small so the downstream engines start early
EXT_CH = [(0, 3), (3, 6), (6, 10), (10, 14)]
INT_CH = [(1, 2), (2, 5), (5, 9), (9, 13)]  # interior blocks b (ext idx)


def _grid_x():
    # Match the reference grid bit-for-bit: jnp.linspace in fp32 on CPU.
    import jax
    import jax.numpy as jnp
    DX = np.float32(1.0 / (RES / 2 - 1))

    def _ls():
        return jnp.linspace(
            DX * (1 - RES / 2), DX * (RES / 2 - 1), RES, dtype=jnp.float32
        )

    try:
        with jax.default_device(jax.devices("cpu")[0]):
            x = np.asarray(_ls())
    except Exception:
        x = np.asarray(_ls())
    return x


def _build_fdm():
    """Shared [96, 96] bf16 a1-FD lhsT: fdm[q, p] = coeff of source a1=q
    for output a1=p; 0.5 Christoffel folded in. +-11.75 / +-23.5, exact
    in bf16."""
    import ml_dtypes
    # FD entries are divided by C_INT (+-1 / +-2, exact in bf16); the
    # identity block carries 1/C_INT so hc = recip(psh) = C_INT * psi^-4
    # comes out pre-scaled and every W mul is a plain tensor_tensor.
    m = np.zeros((RES, 2 * RES), np.float64)
    for p in range(RES):
        if p == 0:
            m[1, p] = 2.0
            m[0, p] = -2.0
        elif p == RES - 1:
            m[RES - 1, p] = 2.0
            m[RES - 2, p] = -2.0
        else:
            m[p + 1, p] = 1.0
            m[p - 1, p] = -1.0
        m[p, RES + p] = 1.0 / C_INT
    return m.astype(ml_dtypes.bfloat16)


def _build_program():
    import dataclasses as _dc

    import concourse.bacc as bacc
    import concourse.mybir as mybir
    import concourse.tile as tile
    from concourse.alu_op_type import AluOpType

    DT = mybir.dt.float32
    BF = mybir.dt.bfloat16
    AF = mybir.ActivationFunctionType

    def view(apv, off, dims):
        return _dc.replace(apv, offset=apv.offset + off, ap=[apv.ap[0]] + dims)

    QP = 512                       # per-BH stride in a chunk's psum tile
    # rsqb region start per ext chunk 1..3: [BH0 padded to 512 | BH1 W]
    RB_OFF = {}
    _o = 0
    for _ci in range(1, 4):
        RB_OFF[_ci] = _o
        _o += QP + (EXT_CH[_ci][1] - EXT_CH[_ci][0]) * RES
    RB_TOT = _o

    nc = bacc.Bacc(None, target_bir_lowering=False, debug=True)
    HW0 = 2 * RES + 2 * EXT_CH[0][1] * RES      # lhsT + chunk-0 rhs, both BHs
    d_rsqa = nc.dram_tensor("rsqa", [2, HW0], BF, kind="ExternalInput")
    d_rsqb = nc.dram_tensor("rsqb", [2, RB_TOT], BF, kind="ExternalInput")
    d_fdm = nc.dram_tensor("fdm", [RES, 2 * RES], BF, kind="ExternalInput")
    d_kb = nc.dram_tensor("kb", [RES, 3], DT, kind="ExternalInput")
    d_w0 = nc.dram_tensor("w0", [RES, I], BF, kind="ExternalOutput")
    d_w1 = nc.dram_tensor("w1", [RES, I], BF, kind="ExternalOutput")
    d_w2 = nc.dram_tensor("w2", [RES, I], BF, kind="ExternalOutput")

    QP = 512                       # per-BH stride inside a chunk's q/psum tile

    with tile.TileContext(nc) as tc:
        with (
            tc.tile_pool(name="const", bufs=1) as cpool,
            tc.tile_pool(name="work", bufs=2) as wpool,
            tc.tile_pool(name="psum", bufs=2, space="PSUM") as pspool,
            tc.tile_pool(name="psfd", bufs=2, space="PSUM") as fdpool,
            tc.tile_pool(name="psh", bufs=2, space="PSUM") as hpool,
        ):
            # rsq layout: [lhsT(192) | rhs-chunk0 of BH0 | BH1 | full rhs]
            rsq = cpool.tile([2, HW0 + RB_TOT], BF)
            nc.sync.dma_start(rsq[:, 0:HW0], d_rsqa[:])
            nc.sync.dma_start(rsq[:, HW0:HW0 + RB_TOT], d_rsqb[:])
            fdm = cpool.tile([RES, 2 * RES], BF)   # [a1-FD | identity]
            nc.gpsimd.dma_start(fdm[:], d_fdm[:])
            kb = cpool.tile([RES, 3], DT)
            nc.gpsimd.dma_start(kb[:], d_kb[:])

            # ACT table warm-up on a tiny tile, first thing
            dmy = cpool.tile([1, 2], DT)
            nc.vector.memset(dmy[:], 1.0)
            dmy2 = cpool.tile([1, 2], DT)
            nc.scalar.activation(dmy2[:], dmy[:], AF.Abs_reciprocal_sqrt)
            nc.scalar.activation(dmy2[:], dmy[:], AF.Square)



            q = cpool.tile([RES, 2 * E], DT)       # m/(2r), chunk-major
            p2 = cpool.tile([RES, E], DT)          # psi^2
            Hb = cpool.tile([RES, E + 4], BF)      # psi^4 bf16, 2-col pad/side
            hinv = cpool.tile([RES, I], DT)        # psi^-4
            jt = cpool.tile([RES, I], BF)          # a0-FD diff
            kt = cpool.tile([RES, I], BF)          # z-FD diff
            w0t = cpool.tile([RES, I], BF)         # W0 field
            w1t = cpool.tile([RES, I], BF)         # W1 field
            w2t = cpool.tile([RES, I], BF)         # W2 field
            s = cpool.tile([RES, E], DT)           # q0 + q1

            # pad cols of Hb (never contribute to surviving outputs)
            nc.gpsimd.memset(view(Hb[:], 0, [[E + 2, 2], [1, 2]]), 1.0)

            def emit_ext_chunk(ci):
                b0, b1 = EXT_CH[ci]
                W = (b1 - b0) * RES
                lo = b0 * RES
                # r^2/c_n via K=2 matmuls; q = rsqrt per BH straight from PSUM
                for n in range(2):
                    ps = pspool.tile([RES, W], DT, tag=f"ps{n}")
                    if ci == 0:
                        roff = 2 * RES + n * W
                    else:
                        roff = HW0 + RB_OFF[ci] + n * QP
                    nc.tensor.matmul(
                        ps[:],
                        rsq[:, n * RES:(n + 1) * RES],
                        rsq[:, roff:roff + W],
                        start=True, stop=True,
                    )
                    nc.scalar.activation(
                        q[:, 2 * lo + n * W:2 * lo + (n + 1) * W],
                        ps[:], AF.Abs_reciprocal_sqrt,
                    )
                # s = q0 + q1 (DVE)
                nc.vector.tensor_add(
                    s[:, lo:lo + W], q[:, 2 * lo:2 * lo + W],
                    q[:, 2 * lo + W:2 * lo + 2 * W],
                )
                # psi^2 = (s + 1)^2 then h = (psi^2)^2, bf16 (ACT)
                nc.scalar.activation(p2[:, lo:lo + W], s[:, lo:lo + W],
                                     AF.Square, bias=1.0)
                nc.scalar.activation(Hb[:, 2 + lo:2 + lo + W], p2[:, lo:lo + W],
                                     AF.Square)

            def emit_int_chunk(ci):
                b0, b1 = INT_CH[ci]
                nb = b1 - b0
                W = nb * RES
                ilo = (b0 - 1) * RES          # interior col offset (m*96)
                elo = b0 * RES                # ext col offset
                # H fp32 via identity matmul on the idle PE; hinv = 1/H
                psh = hpool.tile([RES, W], DT, tag="psh")
                nc.tensor.matmul(psh[:], fdm[:, RES:2 * RES],
                                 Hb[:, 2 + elo:2 + elo + W],
                                 start=True, stop=True)
                nc.vector.reciprocal_approx_fast(hinv[:, ilo:ilo + W], psh[:])
                # a1-FD matmul
                psf = fdpool.tile([RES, W], DT, tag="psf")
                nc.tensor.matmul(psf[:], fdm[:, 0:RES],
                                 Hb[:, 2 + elo:2 + elo + W],
                                 start=True, stop=True)
                # J = Hb(b+1) - Hb(b-1); K = Hb(z+1) - Hb(z-1) (both DVE 2x)
                nc.vector.tensor_sub(
                    jt[:, ilo:ilo + W],
                    Hb[:, 2 + elo + RES:2 + elo + RES + W],
                    Hb[:, 2 + elo - RES:2 + elo - RES + W],
                )
                nc.gpsimd.tensor_sub(
                    kt[:, ilo:ilo + W],
                    Hb[:, 2 + elo + 1:2 + elo + 1 + W],
                    Hb[:, 2 + elo - 1:2 + elo - 1 + W],
                )
                # W1 = psf * hc (DVE, reads PSUM); W0 = J * hc (gpsimd);
                # W2 = K * hc (gpsimd) -- all contiguous dsts
                hvc = hinv[:, ilo:ilo + W]
                nc.vector.tensor_mul(w1t[:, ilo:ilo + W], psf[:], hvc)
                nc.vector.tensor_mul(w0t[:, ilo:ilo + W], jt[:, ilo:ilo + W], hvc)
                nc.gpsimd.tensor_mul(w2t[:, ilo:ilo + W], kt[:, ilo:ilo + W], hvc)
                # a0 edge-block rescale (per-core kb): chunk 0 -> m=0,
                # chunk 2 -> m=11
                if ci == 0 or ci == 3:
                    eb = 0 if ci == 0 else I - RES
                    col = 0 if ci == 0 else 1
                    hs = wpool.tile([RES, RES], DT, tag="hs")
                    nc.gpsimd.tensor_mul(
                        hs[:], hinv[:, eb:eb + RES],
                        view(kb[:], col, [[0, RES]]),
                    )
                    nc.vector.tensor_mul(
                        w0t[:, eb:eb + RES], jt[:, eb:eb + RES], hs[:],
                    )

            def emit_zedge(mlo, mhi):
                # one-sided z-edge fix for interior blocks [mlo, mhi):
                # W2[z=0] = 2*(Hb(z1)-Hb(z0))*hc, W2[z=95] = 2*(Hb(z95)-Hb(z94))*hc
                nm = mhi - mlo
                ilo = mlo * RES
                elo = (mlo + 1) * RES
                ke = wpool.tile([RES, 2 * nm], BF, tag="ke")
                nc.gpsimd.tensor_sub(
                    _dc.replace(ke[:], ap=[ke[:].ap[0], [2, nm], [1, 2]]),
                    view(Hb[:], 2 + elo + 1, [[RES, nm], [94, 2]]),
                    view(Hb[:], 2 + elo, [[RES, nm], [94, 2]]),
                )
                he = wpool.tile([RES, 2 * nm], DT, tag="he")
                nc.vector.tensor_mul(
                    _dc.replace(he[:], ap=[he[:].ap[0], [2, nm], [1, 2]]),
                    view(hinv[:], ilo, [[RES, nm], [95, 2]]),
                    view(kb[:], 2, [[0, nm], [0, 2]]),
                )
                nc.vector.tensor_mul(
                    view(w2t[:], ilo, [[RES, nm], [95, 2]]),
                    _dc.replace(ke[:], ap=[ke[:].ap[0], [2, nm], [1, 2]]),
                    _dc.replace(he[:], ap=[he[:].ap[0], [2, nm], [1, 2]]),
                )

            emit_ext_chunk(0)
            emit_ext_chunk(1)
            emit_int_chunk(0)
            emit_ext_chunk(2)
            emit_int_chunk(1)
            emit_ext_chunk(3)
            emit_int_chunk(2)
            emit_zedge(0, 8)
            # store the first 8 interior blocks while chunk 3 computes
            SP = 8 * RES
            nc.sync.dma_start(d_w0[:, 0:SP], w0t[:, 0:SP])
            nc.sync.dma_start(d_w1[:, 0:SP], w1t[:, 0:SP])
            nc.sync.dma_start(d_w2[:, 0:SP], w2t[:, 0:SP])
            emit_int_chunk(3)
            emit_zedge(8, 12)
            nc.sync.dma_start(d_w0[:, SP:I], w0t[:, SP:I])
            nc.sync.dma_start(d_w1[:, SP:I], w1t[:, SP:I])
            nc.sync.dma_start(d_w2[:, SP:I], w2t[:, SP:I])

    nc.finalize()
    return nc


_CACHE = {}


def _get_setup():
    if "nc" not in _CACHE:
        _CACHE["x"] = _grid_x()
        _CACHE["fdm"] = _build_fdm()
        _CACHE["nc"] = _build_program()
    return _CACHE["nc"], _CACHE["x"], _CACHE["fdm"]


def _build_inmaps(BH_positions, BH_masses_presoftplus, x):
    import ml_dtypes
    pos = np.asarray(BH_positions, np.float64).reshape(2, 3)
    pre = np.asarray(BH_masses_presoftplus, np.float32)
    masses = np.log1p(np.exp(pre)).astype(np.float64)
    xd = x.astype(np.float64)

    fdm = _CACHE["fdm"]
    in_maps = []
    W0C = (EXT_CH[0][1] - EXT_CH[0][0]) * RES    # chunk-0 rhs width
    for c in range(N_CORES):
        rsqa = np.zeros((2, 2 * RES + 2 * W0C), np.float64)
        rb_tot = sum(512 + (b1 - b0) * RES for b0, b1 in EXT_CH[1:])
        rsqb = np.zeros((2, rb_tot), np.float64)
        for n in range(2):
            cn = (masses[n] / 2.0) ** 2
            # lhsT: row0 = (x(a1)-px)^2/c, row1 = 1
            rsqa[0, n * RES:(n + 1) * RES] = (xd - pos[n, 0]) ** 2 / cn
            rsqa[1, n * RES:(n + 1) * RES] = 1.0
            # rhs: row0 = 1, row1 = ((y(a0)-py)^2 + (z-pz)^2)/c
            b = np.arange(EXTB)
            a0 = np.clip(c * PLANES + b - 1, 0, RES - 1)
            yterm = (xd[a0] - pos[n, 1]) ** 2
            zterm = (xd - pos[n, 2]) ** 2
            val = (yterm[:, None] + zterm[None, :]).reshape(-1) / cn
            rsqa[0, 2 * RES + n * W0C:2 * RES + (n + 1) * W0C] = 1.0
            rsqa[1, 2 * RES + n * W0C:2 * RES + (n + 1) * W0C] = val[0:W0C]
            o = 0
            for ci in range(1, 4):
                cb0, cb1 = EXT_CH[ci]
                cw = (cb1 - cb0) * RES
                seg = val[cb0 * RES:cb1 * RES]
                if n == 0:
                    rsqb[0, o:o + 512] = 1.0
                    rsqb[1, o:o + 512] = 1.0
                    rsqb[1, o:o + cw] = seg
                else:
                    rsqb[0, o + 512:o + 512 + cw] = 1.0
                    rsqb[1, o + 512:o + 512 + cw] = seg
                o += 512 + cw
        kb = np.full((RES, 3), 1.0, np.float32)
        kb[:, 2] = 2.0
        if c == 0:
            kb[:, 0] = 2.0
        if c == N_CORES - 1:
            kb[:, 1] = 2.0
        in_maps.append({
            "rsqa": rsqa.astype(ml_dtypes.bfloat16),
            "rsqb": rsqb.astype(ml_dtypes.bfloat16),
            "fdm": fdm,
            "kb": kb,
        })
    return in_maps


# Gamma^i_{jk} = delta_ij W_k + delta_ik W_j - delta_jk W_i:
# per slot s = 9i+3j+k a list of (field c, sign)
_SLOT_TERMS = []
for _i in range(3):
    for _j in range(3):
        for _k in range(3):
            t = []
            if _i == _j:
                t.append((_k, 1.0))
            if _i == _k:
                t.append((_j, 1.0))
            if _j == _k:
                t.append((_i, -1.0))
            _SLOT_TERMS.append(t)


def kernel(BH_positions, BH_masses_presoftplus):
    from concourse.bass_utils import run_bass_kernel_spmd

    nc, x, fdm = _get_setup()
    in_maps = _build_inmaps(BH_positions, BH_masses_presoftplus, x)
    res = run_bass_kernel_spmd(nc, in_maps, list(range(N_CORES)))

    # host gather: [a1, a0l, c, z] per core -> W[a0, a1, z, c] f32
    parts = np.stack([
        np.stack([
            np.asarray(res.results[c][f"w{k}"]).reshape(RES, PLANES, RES)
            for k in range(3)
        ], axis=-2)
        for c in range(N_CORES)
    ])  # [core, a1, a0l, c, z]
    # exact removal of the bf16(1/C_INT) identity-scale rounding: the device
    # W's all carry a factor 1/(u*C_INT) with u = bf16(1/C_INT)
    import ml_dtypes
    u = float(np.float64(np.array(1.0 / C_INT, dtype=ml_dtypes.bfloat16)))
    W = (parts.astype(np.float32) * np.float32(u * C_INT)).transpose(
        0, 2, 1, 4, 3
    ).reshape(RES, RES, RES, 3)  # [a0, a1, z, c]
    out = np.zeros((RES, RES, RES, 27), np.float32)
    for s, terms in enumerate(_SLOT_TERMS):
        for cfld, sgn in terms:
            if sgn > 0:
                out[..., s] += W[..., cfld]
            else:
                out[..., s] -= W[..., cfld]
    return np.ascontiguousarray(out).reshape(RES, RES, RES, 3, 3, 3)


# revision 56
# speedup vs baseline: 1.0337x; 1.0337x over previous
"""Trainium2 Bass kernel for the Brill-Lindquist Christoffel-symbol grid.

Math: the reference reduces to
    psi  = 1 + sum_n m_n / (2 r_n),   m = softplus(pre)
    h    = psi^4
    G_c  = finite-difference gradient of h along grid axis c (2nd order
           central interior, 1st order one-sided edges, spacing DX)
    W_c  = 0.5 * G_c / h
    Gamma^i_{jk} = delta_ij W_k + delta_ik W_j - delta_jk W_i
so the [96,96,96,3,3,3] output carries only 3 distinct scalar fields
(W_0, W_1, W_2) per point; the 27 slots are +-W_c / 0.

Layout: partitions = a1 (96 of 128), free = (a0_ext, z) with a0 sharded
12 planes/core + 1 clamped halo plane each side (14 blocks x 96 = 1344
ext cols, 1152 interior). Pipeline per a0 chunk (4 chunks, small first
so downstream engines start early):
  - r^2/c_n by K=2 matmuls (PE), q_n = m_n/(2 r_n) via ONE ACT
    Abs_reciprocal_sqrt pass straight from PSUM (~4e-5 accurate,
    replaces the vector reciprocal + sqrt chain);
  - psi^2 = Square(q0+q1, bias=1) and h = Square(psi^2) -> bf16 (ACT);
  - H fp32 re-materialized by an identity matmul of h_bf16 on the idle
    PE (scaled by 1/C_INT), so hc = recip(PSUM) = C_INT * psi^-4 comes
    out pre-scaled on DVE with no extra squaring pass;
  - a1-FD  = ONE shared 96x96 bf16 matmul matrix (edge rows one-sided,
    all scales folded: entries +-1/+-2), W1 = PSUM * hc (DVE);
  - a0-FD  = dense free-dim shift-sub by +-96 cols (bf16, DVE 2x);
  - z-FD   = dense shift-sub by +-1 col; block-boundary cols rewritten
    by a strided one-sided fix on gpsimd (W0/W2 muls also gpsimd).
Clamped ghost planes make the global a0 one-sided edge equal to the
central formula up to a scale, supplied per core as a broadcast input
vector, so ONE program serves all 8 cores (SPMD).

Device output per core is just the 3 distinct fields, bf16, slot-packed
[a1(96), a0_loc(12) x c(3) x z(96)] = 0.66 MB (vs 95.6 MB full f32
output); the host inserts zeros/signs for the 27 slots, upcasts and
permutes, the same class of gather work as the zero-slot insertion the
problem requires anyway.
"""

import numpy as np

RES = 96
N_CORES = 8
PLANES = RES // N_CORES        # 12 a0 planes per core
EXTB = PLANES + 2              # 14 ext blocks (halo both sides)
E = EXTB * RES                 # 1344 ext free cols
I = PLANES * RES               # 1152 interior free cols
OW = PLANES * 3 * RES          # 3456 output cols per partition

_DX = float(np.float32(1.0 / (RES / 2 - 1)))
C_INT = 0.25 / _DX             # 11.75  (0.5 christoffel * central 1/(2DX))
C_EDG = 0.5 / _DX              # 23.5   (0.5 christoffel * one-sided 1/DX)

# ext chunks (blocks) for the field build and matching interior chunks;
# first chunk small so the downstream engines start early
EXT_CH = [(0, 3), (3, 6), (6, 10), (10, 14)]
INT_CH = [(1, 2), (2, 5), (5, 9), (9, 13)]  # interior blocks b (ext idx)


def _grid_x():
    # Match the reference grid bit-for-bit: jnp.linspace in fp32 on CPU.
    import jax
    import jax.numpy as jnp
    DX = np.float32(1.0 / (RES / 2 - 1))

    def _ls():
        return jnp.linspace(
            DX * (1 - RES / 2), DX * (RES / 2 - 1), RES, dtype=jnp.float32
        )

    try:
        with jax.default_device(jax.devices("cpu")[0]):
            x = np.asarray(_ls())
    except Exception:
        x = np.asarray(_ls())
    return x


def _build_fdm():
    """Shared [96, 96] bf16 a1-FD lhsT: fdm[q, p] = coeff of source a1=q
    for output a1=p; 0.5 Christoffel folded in. +-11.75 / +-23.5, exact
    in bf16."""
    import ml_dtypes
    # FD entries are divided by C_INT (+-1 / +-2, exact in bf16); the
    # identity block carries 1/C_INT so hc = recip(psh) = C_INT * psi^-4
    # comes out pre-scaled and every W mul is a plain tensor_tensor.
    m = np.zeros((RES, 2 * RES), np.float64)
    for p in range(RES):
        if p == 0:
            m[1, p] = 2.0
            m[0, p] = -2.0
        elif p == RES - 1:
            m[RES - 1, p] = 2.0
            m[RES - 2, p] = -2.0
        else:
            m[p + 1, p] = 1.0
            m[p - 1, p] = -1.0
        m[p, RES + p] = 1.0 / C_INT
    return m.astype(ml_dtypes.bfloat16)


def _build_program():
    import dataclasses as _dc

    import concourse.bacc as bacc
    import concourse.mybir as mybir
    import concourse.tile as tile
    from concourse.alu_op_type import AluOpType

    DT = mybir.dt.float32
    BF = mybir.dt.bfloat16
    AF = mybir.ActivationFunctionType

    def view(apv, off, dims):
        return _dc.replace(apv, offset=apv.offset + off, ap=[apv.ap[0]] + dims)

    QP = 512                       # per-BH stride in a chunk's psum tile
    # rsqb region start per ext chunk 1..3: [BH0 padded to 512 | BH1 W]
    RB_OFF = {}
    _o = 0
    for _ci in range(1, 4):
        RB_OFF[_ci] = _o
        _o += QP + (EXT_CH[_ci][1] - EXT_CH[_ci][0]) * RES
    RB_TOT = _o

    nc = bacc.Bacc(None, target_bir_lowering=False, debug=True)
    HW0 = 2 * RES + 2 * EXT_CH[0][1] * RES      # lhsT + chunk-0 rhs, both BHs
    d_rsqa = nc.dram_tensor("rsqa", [2, HW0], BF, kind="ExternalInput")
    d_rsqb = nc.dram_tensor("rsqb", [2, RB_TOT], BF, kind="ExternalInput")
    d_fdm = nc.dram_tensor("fdm", [RES, 2 * RES], BF, kind="ExternalInput")
    d_kb = nc.dram_tensor("kb", [RES, 3], DT, kind="ExternalInput")
    d_w0 = nc.dram_tensor("w0", [RES, I], BF, kind="ExternalOutput")
    d_w1 = nc.dram_tensor("w1", [RES, I], BF, kind="ExternalOutput")
    d_w2 = nc.dram_tensor("w2", [RES, I], BF, kind="ExternalOutput")

    QP = 512                       # per-BH stride inside a chunk's q/psum tile

    with tile.TileContext(nc) as tc:
        with (
            tc.tile_pool(name="const", bufs=1) as cpool,
            tc.tile_pool(name="work", bufs=2) as wpool,
            tc.tile_pool(name="psum", bufs=2, space="PSUM") as pspool,
            tc.tile_pool(name="psfd", bufs=2, space="PSUM") as fdpool,
            tc.tile_pool(name="psh", bufs=2, space="PSUM") as hpool,
        ):
            # rsq layout: [lhsT(192) | rhs-chunk0 of BH0 | BH1 | full rhs]
            rsq = cpool.tile([2, HW0 + RB_TOT], BF)
            nc.sync.dma_start(rsq[:, 0:HW0], d_rsqa[:])
            nc.sync.dma_start(rsq[:, HW0:HW0 + RB_TOT], d_rsqb[:])
            fdm = cpool.tile([RES, 2 * RES], BF)   # [a1-FD | identity]
            nc.gpsimd.dma_start(fdm[:], d_fdm[:])
            kb = cpool.tile([RES, 3], DT)
            nc.gpsimd.dma_start(kb[:], d_kb[:])

            # ACT table warm-up on a tiny tile, first thing
            dmy = cpool.tile([1, 2], DT)
            nc.vector.memset(dmy[:], 1.0)
            dmy2 = cpool.tile([1, 2], DT)
            nc.scalar.activation(dmy2[:], dmy[:], AF.Abs_reciprocal_sqrt)
            nc.scalar.activation(dmy2[:], dmy[:], AF.Square)



            q = cpool.tile([RES, 2 * E], DT)       # m/(2r), chunk-major
            p2 = cpool.tile([RES, E], DT)          # psi^2
            Hb = cpool.tile([RES, E + 4], BF)      # psi^4 bf16, 2-col pad/side
            hinv = cpool.tile([RES, I], DT)        # psi^-4
            jt = cpool.tile([RES, I], BF)          # a0-FD diff
            kt = cpool.tile([RES, I], BF)          # z-FD diff
            w0t = cpool.tile([RES, I], BF)         # W0 field
            w1t = cpool.tile([RES, I], BF)         # W1 field
            w2t = cpool.tile([RES, I], BF)         # W2 field
            s = cpool.tile([RES, E], DT)           # q0 + q1

            # pad cols of Hb (never contribute to surviving outputs)
            nc.gpsimd.memset(view(Hb[:], 0, [[E + 2, 2], [1, 2]]), 1.0)

            def emit_ext_chunk(ci):
                b0, b1 = EXT_CH[ci]
                W = (b1 - b0) * RES
                lo = b0 * RES
                # r^2/c_n via K=2 matmuls; q = rsqrt per BH straight from PSUM
                for n in range(2):
                    ps = pspool.tile([RES, W], DT, tag=f"ps{n}")
                    if ci == 0:
                        roff = 2 * RES + n * W
                    else:
                        roff = HW0 + RB_OFF[ci] + n * QP
                    nc.tensor.matmul(
                        ps[:],
                        rsq[:, n * RES:(n + 1) * RES],
                        rsq[:, roff:roff + W],
                        start=True, stop=True,
                    )
                    nc.scalar.activation(
                        q[:, 2 * lo + n * W:2 * lo + (n + 1) * W],
                        ps[:], AF.Abs_reciprocal_sqrt,
                    )
                # s = q0 + q1 (DVE)
                nc.vector.tensor_add(
                    s[:, lo:lo + W], q[:, 2 * lo:2 * lo + W],
                    q[:, 2 * lo + W:2 * lo + 2 * W],
                )
                # psi^2 = (s + 1)^2 then h = (psi^2)^2, bf16 (ACT)
                nc.scalar.activation(p2[:, lo:lo + W], s[:, lo:lo + W],
                                     AF.Square, bias=1.0)
                nc.scalar.activation(Hb[:, 2 + lo:2 + lo + W], p2[:, lo:lo + W],
                                     AF.Square)

            def emit_int_chunk(ci):
                b0, b1 = INT_CH[ci]
                nb = b1 - b0
                W = nb * RES
                ilo = (b0 - 1) * RES          # interior col offset (m*96)
                elo = b0 * RES                # ext col offset
                # H fp32 via identity matmul on the idle PE; hinv = 1/H
                psh = hpool.tile([RES, W], DT, tag="psh")
                nc.tensor.matmul(psh[:], fdm[:, RES:2 * RES],
                                 Hb[:, 2 + elo:2 + elo + W],
                                 start=True, stop=True)
                nc.vector.reciprocal_approx_fast(hinv[:, ilo:ilo + W], psh[:])
                # a1-FD matmul
                psf = fdpool.tile([RES, W], DT, tag="psf")
                nc.tensor.matmul(psf[:], fdm[:, 0:RES],
                                 Hb[:, 2 + elo:2 + elo + W],
                                 start=True, stop=True)
                # J = Hb(b+1) - Hb(b-1); K = Hb(z+1) - Hb(z-1) (both DVE 2x)
                nc.vector.tensor_sub(
                    jt[:, ilo:ilo + W],
                    Hb[:, 2 + elo + RES:2 + elo + RES + W],
                    Hb[:, 2 + elo - RES:2 + elo - RES + W],
                )
                nc.vector.tensor_sub(
                    kt[:, ilo:ilo + W],
                    Hb[:, 2 + elo + 1:2 + elo + 1 + W],
                    Hb[:, 2 + elo - 1:2 + elo - 1 + W],
                )
                # W1 = psf * hc (DVE, reads PSUM); W0 = J * hc (gpsimd);
                # W2 = K * hc (gpsimd) -- all contiguous dsts
                hvc = hinv[:, ilo:ilo + W]
                nc.vector.tensor_mul(w1t[:, ilo:ilo + W], psf[:], hvc)
                nc.vector.tensor_mul(w0t[:, ilo:ilo + W], jt[:, ilo:ilo + W], hvc)
                nc.gpsimd.tensor_mul(w2t[:, ilo:ilo + W], kt[:, ilo:ilo + W], hvc)
                # a0 edge-block rescale (per-core kb): chunk 0 -> m=0,
                # chunk 2 -> m=11
                if ci == 0 or ci == 3:
                    eb = 0 if ci == 0 else I - RES
                    col = 0 if ci == 0 else 1
                    hs = wpool.tile([RES, RES], DT, tag="hs")
                    nc.gpsimd.tensor_mul(
                        hs[:], hinv[:, eb:eb + RES],
                        view(kb[:], col, [[0, RES]]),
                    )
                    nc.vector.tensor_mul(
                        w0t[:, eb:eb + RES], jt[:, eb:eb + RES], hs[:],
                    )

            def emit_zedge(mlo, mhi):
                # one-sided z-edge fix for interior blocks [mlo, mhi):
                # W2[z=0] = 2*(Hb(z1)-Hb(z0))*hc, W2[z=95] = 2*(Hb(z95)-Hb(z94))*hc
                nm = mhi - mlo
                ilo = mlo * RES
                elo = (mlo + 1) * RES
                ke = wpool.tile([RES, 2 * nm], BF, tag="ke")
                nc.gpsimd.tensor_sub(
                    _dc.replace(ke[:], ap=[ke[:].ap[0], [2, nm], [1, 2]]),
                    view(Hb[:], 2 + elo + 1, [[RES, nm], [94, 2]]),
                    view(Hb[:], 2 + elo, [[RES, nm], [94, 2]]),
                )
                he = wpool.tile([RES, 2 * nm], DT, tag="he")
                nc.vector.tensor_mul(
                    _dc.replace(he[:], ap=[he[:].ap[0], [2, nm], [1, 2]]),
                    view(hinv[:], ilo, [[RES, nm], [95, 2]]),
                    view(kb[:], 2, [[0, nm], [0, 2]]),
                )
                nc.vector.tensor_mul(
                    view(w2t[:], ilo, [[RES, nm], [95, 2]]),
                    _dc.replace(ke[:], ap=[ke[:].ap[0], [2, nm], [1, 2]]),
                    _dc.replace(he[:], ap=[he[:].ap[0], [2, nm], [1, 2]]),
                )

            emit_ext_chunk(0)
            emit_ext_chunk(1)
            emit_int_chunk(0)
            emit_ext_chunk(2)
            emit_int_chunk(1)
            emit_ext_chunk(3)
            emit_int_chunk(2)
            emit_zedge(0, 8)
            # store the first 8 interior blocks while chunk 3 computes
            SP = 8 * RES
            nc.sync.dma_start(d_w0[:, 0:SP], w0t[:, 0:SP])
            nc.sync.dma_start(d_w1[:, 0:SP], w1t[:, 0:SP])
            nc.sync.dma_start(d_w2[:, 0:SP], w2t[:, 0:SP])
            emit_int_chunk(3)
            emit_zedge(8, 12)
            nc.sync.dma_start(d_w0[:, SP:I], w0t[:, SP:I])
            nc.sync.dma_start(d_w1[:, SP:I], w1t[:, SP:I])
            nc.sync.dma_start(d_w2[:, SP:I], w2t[:, SP:I])

    nc.finalize()
    return nc


_CACHE = {}


def _get_setup():
    if "nc" not in _CACHE:
        _CACHE["x"] = _grid_x()
        _CACHE["fdm"] = _build_fdm()
        _CACHE["nc"] = _build_program()
    return _CACHE["nc"], _CACHE["x"], _CACHE["fdm"]


def _build_inmaps(BH_positions, BH_masses_presoftplus, x):
    import ml_dtypes
    pos = np.asarray(BH_positions, np.float64).reshape(2, 3)
    pre = np.asarray(BH_masses_presoftplus, np.float32)
    masses = np.log1p(np.exp(pre)).astype(np.float64)
    xd = x.astype(np.float64)

    fdm = _CACHE["fdm"]
    in_maps = []
    W0C = (EXT_CH[0][1] - EXT_CH[0][0]) * RES    # chunk-0 rhs width
    for c in range(N_CORES):
        rsqa = np.zeros((2, 2 * RES + 2 * W0C), np.float64)
        rb_tot = sum(512 + (b1 - b0) * RES for b0, b1 in EXT_CH[1:])
        rsqb = np.zeros((2, rb_tot), np.float64)
        for n in range(2):
            cn = (masses[n] / 2.0) ** 2
            # lhsT: row0 = (x(a1)-px)^2/c, row1 = 1
            rsqa[0, n * RES:(n + 1) * RES] = (xd - pos[n, 0]) ** 2 / cn
            rsqa[1, n * RES:(n + 1) * RES] = 1.0
            # rhs: row0 = 1, row1 = ((y(a0)-py)^2 + (z-pz)^2)/c
            b = np.arange(EXTB)
            a0 = np.clip(c * PLANES + b - 1, 0, RES - 1)
            yterm = (xd[a0] - pos[n, 1]) ** 2
            zterm = (xd - pos[n, 2]) ** 2
            val = (yterm[:, None] + zterm[None, :]).reshape(-1) / cn
            rsqa[0, 2 * RES + n * W0C:2 * RES + (n + 1) * W0C] = 1.0
            rsqa[1, 2 * RES + n * W0C:2 * RES + (n + 1) * W0C] = val[0:W0C]
            o = 0
            for ci in range(1, 4):
                cb0, cb1 = EXT_CH[ci]
                cw = (cb1 - cb0) * RES
                seg = val[cb0 * RES:cb1 * RES]
                if n == 0:
                    rsqb[0, o:o + 512] = 1.0
                    rsqb[1, o:o + 512] = 1.0
                    rsqb[1, o:o + cw] = seg
                else:
                    rsqb[0, o + 512:o + 512 + cw] = 1.0
                    rsqb[1, o + 512:o + 512 + cw] = seg
                o += 512 + cw
        kb = np.full((RES, 3), 1.0, np.float32)
        kb[:, 2] = 2.0
        if c == 0:
            kb[:, 0] = 2.0
        if c == N_CORES - 1:
            kb[:, 1] = 2.0
        in_maps.append({
            "rsqa": rsqa.astype(ml_dtypes.bfloat16),
            "rsqb": rsqb.astype(ml_dtypes.bfloat16),
            "fdm": fdm,
            "kb": kb,
        })
    return in_maps


# Gamma^i_{jk} = delta_ij W_k + delta_ik W_j - delta_jk W_i:
# per slot s = 9i+3j+k a list of (field c, sign)
_SLOT_TERMS = []
for _i in range(3):
    for _j in range(3):
        for _k in range(3):
            t = []
            if _i == _j:
                t.append((_k, 1.0))
            if _i == _k:
                t.append((_j, 1.0))
            if _j == _k:
                t.append((_i, -1.0))
            _SLOT_TERMS.append(t)


def kernel(BH_positions, BH_masses_presoftplus):
    from concourse.bass_utils import run_bass_kernel_spmd

    nc, x, fdm = _get_setup()
    in_maps = _build_inmaps(BH_positions, BH_masses_presoftplus, x)
    res = run_bass_kernel_spmd(nc, in_maps, list(range(N_CORES)))

    # host gather: [a1, a0l, c, z] per core -> W[a0, a1, z, c] f32
    parts = np.stack([
        np.stack([
            np.asarray(res.results[c][f"w{k}"]).reshape(RES, PLANES, RES)
            for k in range(3)
        ], axis=-2)
        for c in range(N_CORES)
    ])  # [core, a1, a0l, c, z]
    # exact removal of the bf16(1/C_INT) identity-scale rounding: the device
    # W's all carry a factor 1/(u*C_INT) with u = bf16(1/C_INT)
    import ml_dtypes
    u = float(np.float64(np.array(1.0 / C_INT, dtype=ml_dtypes.bfloat16)))
    W = (parts.astype(np.float32) * np.float32(u * C_INT)).transpose(
        0, 2, 1, 4, 3
    ).reshape(RES, RES, RES, 3)  # [a0, a1, z, c]
    out = np.zeros((RES, RES, RES, 27), np.float32)
    for s, terms in enumerate(_SLOT_TERMS):
        for cfld, sgn in terms:
            if sgn > 0:
                out[..., s] += W[..., cfld]
            else:
                out[..., s] -= W[..., cfld]
    return np.ascontiguousarray(out).reshape(RES, RES, RES, 3, 3, 3)
